# revision 3
# baseline (speedup 1.0000x reference)
"""PointNet++ instance-seg feature kernel on 8 trn2 NeuronCores.

Strategy: batch-parallel (4 point clouds per NeuronCore). The per-(sample,k)
gather is done with gpsimd ap_gather over a "doubled" 32768-row table per
batch: rows [0,16384) hold the masked-in channel set (f, 1, f_xyz, unit f_xyz,
unit f_nrm), rows [16384,32768) zero the masked channels. Host adds
16384*(dist > 0.05) to each index so one gather stream yields both masked and
unmasked partial sums. Segment-64 sums via DVE tensor_reduce, PE transpose to
sample-major, small DVE finalize (discrepancy / cosine sims / cross product).
"""
import os
import sys
import types
from contextlib import ExitStack

import numpy as np

import concourse.bass as bass
import concourse.tile as tile
from concourse import bacc, mybir, bass_utils

# NTFF profile hook (container's antenv lacks axon_hooks); harmless if unused.
try:
    from trn_agent_boot.trn_boot import _ntff_profile_via_ctypes
    _m = types.ModuleType("antenv.axon_hooks")
    _hook = _ntff_profile_via_ctypes('/opt/axon/libaxon_pjrt.so')
    _m.get_axon_ntff_profile_hook = lambda: _hook
    sys.modules.setdefault("antenv.axon_hooks", _m)
    bass_utils.upload_artifacts = lambda tmpdir: f"local:{tmpdir}"
except Exception:
    pass

BZ, N, NSMP, K, C = 32, 16384, 4096, 64, 6
NC_COUNT = 8
B_PER_NC = BZ // NC_COUNT          # 4 batches per core
NE2 = 2 * N                        # doubled table rows
NIDX = 2048                        # idxs per ap_gather call (per group)
NCHUNK = (2048 * K) // NIDX        # 64 calls; each group owns 2048 samples
R_DISC = np.float32(0.05)

_COMPILED = None
LAST_EXEC_NS = None


def _build_nc():
    nc = bacc.Bacc("TRN2", target_bir_lowering=False, debug=False,
                   enable_asserts=False, num_devices=NC_COUNT)
    table_d = nc.dram_tensor("table", (128, NE2, 1), mybir.dt.float32,
                             kind="ExternalInput").ap()
    idxw_d = nc.dram_tensor("idxw", (128, 8192), mybir.dt.int16,
                            kind="ExternalInput").ap()
    ct_d = nc.dram_tensor("ctile", (128, 16, 8, 16), mybir.dt.float32,
                          kind="ExternalInput").ap()
    id_d = nc.dram_tensor("ident", (128, 128), mybir.dt.float32,
                          kind="ExternalInput").ap()
    out_d = nc.dram_tensor("out", (B_PER_NC, NSMP, 11), mybir.dt.float32,
                           kind="ExternalOutput").ap()

    with tile.TileContext(nc) as tc, ExitStack() as ctx:
        _emit(ctx, tc, out_d, table_d, idxw_d, ct_d, id_d)
    nc.compile()
    return nc


def _emit(ctx, tc, out_d, table_d, idxw_d, ct_d, id_d):
    nc = tc.nc
    f32 = mybir.dt.float32
    X = mybir.AxisListType.X
    ADD = mybir.AluOpType.add

    pool = ctx.enter_context(tc.tile_pool(name="main", bufs=1))
    tt = pool.tile([128, NE2, 1], f32)
    for i in range(16):
        nc.sync.dma_start(tt[8 * i:8 * i + 8], table_d[8 * i:8 * i + 8])
    it = pool.tile([128, 8192], mybir.dt.int16)
    for i in range(4):
        nc.sync.dma_start(it[32 * i:32 * i + 32], idxw_d[32 * i:32 * i + 32])
    ct = pool.tile([128, 16, 8, 16], f32)
    nc.sync.dma_start(ct[:], ct_d)
    idm = pool.tile([128, 128], f32)
    nc.sync.dma_start(idm[:], id_d)

    sums = pool.tile([128, 2048], f32)
    out_sb = pool.tile([128, 16, 8, 11], f32)
    scr = pool.tile([128, 8, 4], f32)

    gpool = ctx.enter_context(tc.tile_pool(name="g", bufs=2))
    ppool = ctx.enter_context(tc.psum_pool(name="ps", bufs=2))

    ncol = NIDX // 16
    spc = NIDX // K                  # samples covered per chunk per group
    for ci in range(NCHUNK):
        gout = gpool.tile([128, NIDX, 1], f32)
        nc.gpsimd.ap_gather(gout[:], tt[:], it[:, ncol * ci:ncol * (ci + 1)],
                            channels=128, num_elems=NE2, d=1, num_idxs=NIDX)
        gv = gout[:].rearrange("p (s k) d -> p s (k d)", k=K)
        nc.vector.tensor_reduce(sums[:, spc * ci:spc * (ci + 1)], gv,
                                axis=X, op=ADD)

    # cross-product index pairs: out_ch -> (ct_ch_a, v_ch_b, ct_ch_c, v_ch_d)
    cross_map = {8: (13, 9, 14, 8), 9: (14, 7, 12, 9), 10: (12, 8, 13, 7)}
    for B in range(16):
        tp = ppool.tile([128, 128], f32)
        nc.tensor.transpose(tp[:], sums[:, 128 * B:128 * (B + 1)], idm[:])
        v = tp[:].rearrange("p (g c) -> p g c", g=8)
        cb = ct[:, B]
        ob = out_sb[:, B]
        t0 = scr[:, :, 0]
        t1 = scr[:, :, 1]
        rec = cb[:, :, 15]
        for c in range(6):
            nc.vector.tensor_mul(t0, v[:, :, c], rec)
            nc.vector.tensor_sub(ob[:, :, c], cb[:, :, c], t0)
        dt = scr[:, :, 0:3]
        nc.vector.tensor_mul(dt, cb[:, :, 6:9], v[:, :, 10:13])
        nc.vector.tensor_reduce(ob[:, :, 6], dt, axis=X, op=ADD)
        nc.vector.tensor_mul(dt, cb[:, :, 9:12], v[:, :, 13:16])
        nc.vector.tensor_reduce(ob[:, :, 7], dt, axis=X, op=ADD)
        for oc, (a, b2, c2, d2) in cross_map.items():
            nc.vector.tensor_mul(t0, cb[:, :, a], v[:, :, b2])
            nc.vector.tensor_mul(t1, cb[:, :, c2], v[:, :, d2])
            nc.vector.tensor_sub(ob[:, :, oc], t0, t1)

    for g in range(8):
        b, h = g // 2, g % 2
        dv = out_d[b, 2048 * h:2048 * (h + 1), :].rearrange(
            "(B j) c -> j B c", j=128)
        nc.sync.dma_start(dv, out_sb[:, :, g, :])


def _prep_inputs(features, idx, dist, fps):
    f32 = np.float32
    f = np.ascontiguousarray(features, dtype=f32)

    nrm1 = np.linalg.norm(f[:, :, 0:3], axis=2, keepdims=True)
    nrm2 = np.linalg.norm(f[:, :, 3:6], axis=2, keepdims=True)
    n1 = f[:, :, 0:3] / nrm1
    n2 = f[:, :, 3:6] / nrm2
    ft = f.transpose(0, 2, 1)
    low = np.concatenate([
        ft,
        np.ones((BZ, 1, N), f32),
        ft[:, 0:3],
        n1.transpose(0, 2, 1),
        n2.transpose(0, 2, 1),
    ], axis=1).astype(f32)                     # [32, 16, 16384]
    high = low.copy()
    high[:, 0:7] = 0.0
    T = np.concatenate([low, high], axis=2)    # [32, 16, 32768]

    bidx = 4 * np.arange(8)[:, None] + np.arange(8)[None, :] // 2   # [nc, g]
    hidx = np.tile(np.arange(8) % 2, (8, 1))                        # [nc, g]
    tables = T[bidx].reshape(8, 128, NE2, 1)   # [nc, 128, 32768, 1]

    mask_out = (np.asarray(dist, dtype=f32) > R_DISC)
    idxd = (np.asarray(idx, dtype=np.int32) + NE2 // 2 * mask_out
            ).astype(np.int16)                 # [32, 4096, 64]
    st = idxd.reshape(BZ, 2, 2048 * K)
    streams = st[bidx, hidx]                   # [8, 8, 131072]
    idxw = streams.reshape(8, 8, 8192, 16).transpose(0, 1, 3, 2) \
                  .reshape(8, 128, 8192).copy()

    c_ = f.reshape(-1, C)[np.asarray(fps, dtype=np.int64)].reshape(BZ, NSMP, C)
    M = np.sum(~mask_out, axis=2)
    with np.errstate(divide="ignore"):
        rec = (1.0 / M).astype(f32)
    cn1 = c_[:, :, 0:3]
    cn2 = c_[:, :, 3:6]
    u = (cn1 / np.linalg.norm(cn1, axis=2, keepdims=True) / K).astype(f32)
    v = (cn2 / np.linalg.norm(cn2, axis=2, keepdims=True) / K).astype(f32)
    w = (cn1 / K).astype(f32)
    cc = np.concatenate([c_, u, v, w, rec[..., None]], axis=2)  # [32,4096,16]
    ccr = cc.reshape(BZ, 2, 16, 128, 16)       # [b, h, B, j, 16]
    ctile = ccr[bidx, hidx].transpose(0, 3, 2, 1, 4).copy()  # [8,128,16,8,16]

    ident = np.eye(128, dtype=f32)
    in_maps = []
    for n in range(8):
        in_maps.append({
            "table": np.ascontiguousarray(tables[n]),
            "idxw": np.ascontiguousarray(idxw[n]),
            "ctile": np.ascontiguousarray(ctile[n]),
            "ident": ident,
        })
    return in_maps


def kernel(**inputs):
    global _COMPILED, LAST_EXEC_NS
    features = np.asarray(inputs["features"])
    idx = np.asarray(inputs["nearest_k_idx"])
    dist = np.asarray(inputs["nearest_k_dist"])
    fps = np.asarray(inputs["fps_idx"])

    if _COMPILED is None:
        _COMPILED = _build_nc()
    nc = _COMPILED

    in_maps = _prep_inputs(features, idx, dist, fps)
    trace = os.environ.get("KERNEL_TRACE") == "1"
    res = bass_utils.run_bass_kernel_spmd(nc, in_maps,
                                          core_ids=list(range(NC_COUNT)),
                                          trace=trace)
    if trace:
        LAST_EXEC_NS = res.exec_time_ns
        globals()["LAST_RES"] = res
    out = np.concatenate([res.results[n]["out"] for n in range(NC_COUNT)],
                         axis=0)
    return out.astype(np.float32)


# revision 13
# speedup vs baseline: 1.0497x; 1.0497x over previous
"""PointNet++ instance-seg feature kernel on 8 trn2 NeuronCores.

Strategy: batch-parallel (4 point clouds per NeuronCore). The per-(sample,k)
gather is done with gpsimd ap_gather over a "doubled" 32768-row table per
batch: rows [0,16384) hold the masked-in channel set (f, 1, f_xyz, unit f_xyz,
unit f_nrm), rows [16384,32768) zero the masked channels. Host adds
16384*(dist > 0.05) to each index so one gather stream yields both masked and
unmasked partial sums. Segment-64 sums via DVE tensor_reduce, PE transpose to
sample-major, small DVE finalize (discrepancy / cosine sims / cross product).
"""
import os
import sys
import types
from contextlib import ExitStack

import numpy as np

import concourse.bass as bass
import concourse.tile as tile
from concourse import bacc, mybir, bass_utils

# NTFF profile hook (container's antenv lacks axon_hooks); harmless if unused.
try:
    from trn_agent_boot.trn_boot import _ntff_profile_via_ctypes
    _m = types.ModuleType("antenv.axon_hooks")
    _hook = _ntff_profile_via_ctypes('/opt/axon/libaxon_pjrt.so')
    _m.get_axon_ntff_profile_hook = lambda: _hook
    sys.modules.setdefault("antenv.axon_hooks", _m)
    bass_utils.upload_artifacts = lambda tmpdir: f"local:{tmpdir}"
except Exception:
    pass

BZ, N, NSMP, K, C = 32, 16384, 4096, 64, 6
NC_COUNT = 8
B_PER_NC = BZ // NC_COUNT          # 4 batches per core
NE2 = 2 * N                        # doubled table rows
NIDX = 2048                        # idxs per ap_gather call (per group)
NCHUNK = (2048 * K) // NIDX        # 64 calls; each group owns 2048 samples
R_DISC = np.float32(0.05)

_COMPILED = None
LAST_EXEC_NS = None


def _build_nc():
    nc = bacc.Bacc("TRN2", target_bir_lowering=False, debug=False,
                   enable_asserts=False, num_devices=NC_COUNT)
    table_d = nc.dram_tensor("table", (128, N, 1), mybir.dt.float32,
                             kind="ExternalInput").ap()
    idxw_d = nc.dram_tensor("idxw", (128, 8192), mybir.dt.int16,
                            kind="ExternalInput").ap()
    ct_d = nc.dram_tensor("ctile", (128, 16, 8, 16), mybir.dt.float32,
                          kind="ExternalInput").ap()
    mk_d = nc.dram_tensor("chmask", (128, 1), mybir.dt.float32,
                          kind="ExternalInput").ap()
    id_d = nc.dram_tensor("ident", (128, 128), mybir.dt.float32,
                          kind="ExternalInput").ap()
    out_d = nc.dram_tensor("out", (B_PER_NC, NSMP, 11), mybir.dt.float32,
                           kind="ExternalOutput").ap()

    with tile.TileContext(nc) as tc, ExitStack() as ctx:
        _emit(ctx, tc, out_d, table_d, idxw_d, ct_d, id_d, mk_d)
    nc.compile()
    return nc


def _emit(ctx, tc, out_d, table_d, idxw_d, ct_d, id_d, mk_d):
    nc = tc.nc
    f32 = mybir.dt.float32
    X = mybir.AxisListType.X
    ADD = mybir.AluOpType.add

    pool = ctx.enter_context(tc.tile_pool(name="main", bufs=1))
    eng = [nc.sync, nc.scalar, nc.gpsimd]
    tt = pool.tile([128, NE2, 1], f32)
    mk = pool.tile([128, 1], f32)
    eng[1].dma_start(mk[:], mk_d)
    for i in range(16):
        eng[i % 3].dma_start(tt[8 * i:8 * i + 8, 0:N, :],
                             table_d[8 * i:8 * i + 8])
    # high half = low half * per-partition mask (0 for ch0-6, 1 for ch7-15)
    nc.vector.tensor_scalar_mul(tt[:, N:NE2, :], tt[:, 0:N, :], mk[:])
    it = pool.tile([128, 8192], mybir.dt.int16)
    for i in range(4):
        eng[(16 + i) % 3].dma_start(it[32 * i:32 * i + 32],
                                    idxw_d[32 * i:32 * i + 32])
    ct = pool.tile([128, 16, 8, 16], f32)
    eng[0].dma_start(ct[:], ct_d)
    idm = pool.tile([128, 128], f32)
    eng[1].dma_start(idm[:], id_d)

    sums = pool.tile([128, 2048], f32)
    out_sb = pool.tile([128, 16, 8, 11], f32)
    scr = pool.tile([128, 8, 4], f32)

    gpool = ctx.enter_context(tc.tile_pool(name="g", bufs=2))
    ppool = ctx.enter_context(tc.psum_pool(name="ps", bufs=2))

    ncol = NIDX // 16
    spc = NIDX // K                  # samples covered per chunk per group
    for ci in range(NCHUNK):
        gout = gpool.tile([128, NIDX, 1], f32)
        nc.gpsimd.ap_gather(gout[:], tt[:], it[:, ncol * ci:ncol * (ci + 1)],
                            channels=128, num_elems=NE2, d=1, num_idxs=NIDX)
        gv = gout[:].rearrange("p (s k) d -> p s (k d)", k=K)
        nc.vector.tensor_reduce(sums[:, spc * ci:spc * (ci + 1)], gv,
                                axis=X, op=ADD)

    # cross-product index pairs: out_ch -> (ct_ch_a, v_ch_b, ct_ch_c, v_ch_d)
    cross_map = {8: (13, 9, 14, 8), 9: (14, 7, 12, 9), 10: (12, 8, 13, 7)}
    for B in range(16):
        tp = ppool.tile([128, 128], f32)
        nc.tensor.transpose(tp[:], sums[:, 128 * B:128 * (B + 1)], idm[:])
        v = tp[:].rearrange("p (g c) -> p g c", g=8)
        cb = ct[:, B]
        ob = out_sb[:, B]
        t0 = scr[:, :, 0]
        t1 = scr[:, :, 1]
        rec = cb[:, :, 15]
        for c in range(6):
            nc.vector.tensor_mul(t0, v[:, :, c], rec)
            nc.vector.tensor_sub(ob[:, :, c], cb[:, :, c], t0)
        dt = scr[:, :, 0:3]
        nc.vector.tensor_mul(dt, cb[:, :, 6:9], v[:, :, 10:13])
        nc.vector.tensor_reduce(ob[:, :, 6], dt, axis=X, op=ADD)
        nc.vector.tensor_mul(dt, cb[:, :, 9:12], v[:, :, 13:16])
        nc.vector.tensor_reduce(ob[:, :, 7], dt, axis=X, op=ADD)
        for oc, (a, b2, c2, d2) in cross_map.items():
            nc.vector.tensor_mul(t0, cb[:, :, a], v[:, :, b2])
            nc.vector.tensor_mul(t1, cb[:, :, c2], v[:, :, d2])
            nc.vector.tensor_sub(ob[:, :, oc], t0, t1)

    for g in range(8):
        b, h = g // 2, g % 2
        dv = out_d[b, 2048 * h:2048 * (h + 1), :].rearrange(
            "(B j) c -> j B c", j=128)
        nc.sync.dma_start(dv, out_sb[:, :, g, :])


def _prep_inputs(features, idx, dist, fps):
    f32 = np.float32
    f = np.ascontiguousarray(features, dtype=f32)

    nrm1 = np.linalg.norm(f[:, :, 0:3], axis=2, keepdims=True)
    nrm2 = np.linalg.norm(f[:, :, 3:6], axis=2, keepdims=True)
    n1 = f[:, :, 0:3] / nrm1
    n2 = f[:, :, 3:6] / nrm2
    ft = f.transpose(0, 2, 1)
    low = np.concatenate([
        ft,
        np.ones((BZ, 1, N), f32),
        ft[:, 0:3],
        n1.transpose(0, 2, 1),
        n2.transpose(0, 2, 1),
    ], axis=1).astype(f32)                     # [32, 16, 16384]

    bidx = 4 * np.arange(8)[:, None] + np.arange(8)[None, :] // 2   # [nc, g]
    hidx = np.tile(np.arange(8) % 2, (8, 1))                        # [nc, g]
    tables = low[bidx].reshape(8, 128, N, 1)   # [nc, 128, 16384, 1]

    mask_out = (np.asarray(dist, dtype=f32) > R_DISC)
    idxd = (np.asarray(idx, dtype=np.int32) + NE2 // 2 * mask_out
            ).astype(np.int16)                 # [32, 4096, 64]
    st = idxd.reshape(BZ, 2, 2048 * K)
    streams = st[bidx, hidx]                   # [8, 8, 131072]
    idxw = streams.reshape(8, 8, 8192, 16).transpose(0, 1, 3, 2) \
                  .reshape(8, 128, 8192).copy()

    c_ = f.reshape(-1, C)[np.asarray(fps, dtype=np.int64)].reshape(BZ, NSMP, C)
    M = np.sum(~mask_out, axis=2)
    with np.errstate(divide="ignore"):
        rec = (1.0 / M).astype(f32)
    cn1 = c_[:, :, 0:3]
    cn2 = c_[:, :, 3:6]
    u = (cn1 / np.linalg.norm(cn1, axis=2, keepdims=True) / K).astype(f32)
    v = (cn2 / np.linalg.norm(cn2, axis=2, keepdims=True) / K).astype(f32)
    w = (cn1 / K).astype(f32)
    cc = np.concatenate([c_, u, v, w, rec[..., None]], axis=2)  # [32,4096,16]
    ccr = cc.reshape(BZ, 2, 16, 128, 16)       # [b, h, B, j, 16]
    ctile = ccr[bidx, hidx].transpose(0, 3, 2, 1, 4).copy()  # [8,128,16,8,16]

    ident = np.eye(128, dtype=f32)
    chmask = (np.tile(np.arange(16), 8) >= 7).astype(f32).reshape(128, 1)
    in_maps = []
    for n in range(8):
        in_maps.append({
            "table": np.ascontiguousarray(tables[n]),
            "idxw": np.ascontiguousarray(idxw[n]),
            "ctile": np.ascontiguousarray(ctile[n]),
            "ident": ident,
            "chmask": chmask,
        })
    return in_maps


def kernel(**inputs):
    global _COMPILED, LAST_EXEC_NS
    features = np.asarray(inputs["features"])
    idx = np.asarray(inputs["nearest_k_idx"])
    dist = np.asarray(inputs["nearest_k_dist"])
    fps = np.asarray(inputs["fps_idx"])

    if _COMPILED is None:
        _COMPILED = _build_nc()
    nc = _COMPILED

    in_maps = _prep_inputs(features, idx, dist, fps)
    trace = os.environ.get("KERNEL_TRACE") == "1"
    res = bass_utils.run_bass_kernel_spmd(nc, in_maps,
                                          core_ids=list(range(NC_COUNT)),
                                          trace=trace)
    if trace:
        LAST_EXEC_NS = res.exec_time_ns
        globals()["LAST_RES"] = res
    out = np.concatenate([res.results[n]["out"] for n in range(NC_COUNT)],
                         axis=0)
    return out.astype(np.float32)
